# revision 2
# baseline (speedup 1.0000x reference)
"""Trainium2 Bass kernel for Points3DLoss (robust chamfer loss) — v5.

Computes, for inputs obs (2,16,4096,3) and pred (2,16,2048,3):
  d[bt,n]  = min_m |obs[bt,n] - pred[bt,m]|^2          (chamfer, per frame)
  res      = sqrt(d) reshaped to (B, T*N)
  med, mad = lower-median robust stats per batch row (on detached res)
  w        = bisquare weights; loss = 0.5 * sum(w * res^2)

Strategy (v5):
- Data-parallel over the 32 frames (4 per core).
- PE matmuls in bf16 split precision (hi/lo, K=13) computing
  z' = -0.5*|a-b|^2 directly in PSUM: 4x512-col matmuls per chunk.
- Consumer pipeline per chunk (2048 f32 PSUM cols), ACT-paced at
  ~1.97us/chunk:
    ACT stages the whole chunk -> f16 SBUF in ONE wide copy (sole PSUM
    reader, so PSUM-free depends on this copy alone -- a second PSUM
    reader gets serialized after the copy by Tile's same-tile dep
    coalescing and stalls the pipeline).
    DVE fold tree TT(1024)+TT(512)+TT(256) -> 256-wide per-chunk
    vector, collected per 8 chunks, one batched negated-axis
    TENSOR_REDUCE -> zA cols (chunk max of z').
- d gathered per batch row with four subgroup AllGathers ({0-3},{4-7});
  cols [0:64] after frame 1, [64:96] after frame 2, [96:124] at chunk
  123, [124:128] after the loop (only 2KB exposed at the end).
- Tail: med/mad via 10-round value bisection on fp16 residuals; counts
  split ACT (Sign+accum over half) || DVE (subtract + is_lt cache
  reduce over the other half), combined by one f16 ones-matmul.
  Bisquare weighted sum; host adds core0 + core4 row losses.
"""

import sys

if '/opt/trn_rl_repo' not in sys.path:
    sys.path.insert(0, '/opt/trn_rl_repo')

import numpy as np

B, T, N_OBS, M_PRED = 2, 16, 4096, 2048
BT = B * T
NCORES = 8
F = BT // NCORES          # frames per core = 4
CH = N_OBS // 128         # obs chunks per frame = 32
COLS = F * CH             # d columns per core = 128
NROW = T * N_OBS          # residuals per batch row = 65536
K_MED = 32768.0           # rank (1-based) of lower median
TUNE = 4.6851
MADSTD = 0.67449

ACT_W = 2048              # cols staged by ACT (f16) — whole chunk
H1 = ACT_W // 2           # 1024
TRB = 8                   # chunks per batched tensor_reduce
N_ITERS = 10              # bisection iterations (bracket [0,2*T0])
GROUPS = [[0, 1, 2, 3], [4, 5, 6, 7]]

_CACHE = {}


def _build_nc(stage="D"):
    import concourse.bacc as bacc
    import concourse.tile as tile
    from concourse import mybir
    from contextlib import ExitStack

    A = mybir.AluOpType
    AF = mybir.ActivationFunctionType
    f32 = mybir.dt.float32
    f16 = mybir.dt.float16
    bf16 = mybir.dt.bfloat16
    X = mybir.AxisListType.X

    nc = bacc.Bacc("TRN2", target_bir_lowering=False, debug=False,
                   num_devices=NCORES)

    obs_in = nc.dram_tensor("obs_in", [13, F * N_OBS], bf16,
                            kind="ExternalInput").ap()
    pred_in = nc.dram_tensor("pred_in", [13, F * M_PRED], bf16,
                             kind="ExternalInput").ap()
    out_d = nc.dram_tensor("out", [1, 1], f32, kind="ExternalOutput").ap()

    def emit(tc, pp, stack):
        OBSL = pp.tile([13, F * N_OBS], bf16, name="OBSL", tag="OBSL")
        PREDL = pp.tile([13, F * M_PRED], bf16, name="PREDL", tag="PREDL")
        for f in range(F):
            nc.sync.dma_start(out=PREDL[:, f * M_PRED:(f + 1) * M_PRED],
                              in_=pred_in[:, f * M_PRED:(f + 1) * M_PRED])
            nc.sync.dma_start(out=OBSL[:, f * N_OBS:(f + 1) * N_OBS],
                              in_=obs_in[:, f * N_OBS:(f + 1) * N_OBS])

        zA = pp.tile([128, COLS], f32, name="zA", tag="zA")
        VC = pp.tile([128, TRB, 256], f16, name="VC", tag="VC")
        g = pp.tile([128, 512], f32, name="g", tag="g")

        dp = stack.enter_context(tc.tile_pool(name="dram", bufs=1,
                                              space="DRAM"))
        # three gathers: zA cols [0:64] after frame 1, [64:96] after
        # frame 2, [96:128] after the loop (only 16KB exposed at the end).
        GATHERS = [(0, 64), (64, 32), (96, 28), (124, 4)]
        cc_in = []
        cc_out = []
        for h, (lo, w) in enumerate(GATHERS):
            cc_in.append(dp.tile([128, w], f32, name=f"cc_in{h}"))
            cc_out.append(dp.tile([4, 128, w], f32, name=f"cc_out{h}"))

        def gather_part(h):
            lo, w = GATHERS[h]
            dh = pp.tile([128, w], f32, name=f"dh{h}", tag=f"dh{h}")
            nc.scalar.activation(out=dh, in_=zA[:, lo:lo + w],
                                 func=AF.Relu, bias=0.0, scale=-2.0)
            nc.sync.dma_start(out=cc_in[h], in_=dh)
            nc.gpsimd.collective_compute(
                "AllGather", A.bypass, replica_groups=GROUPS,
                ins=[cc_in[h][:]], outs=[cc_out[h][:]])
            nc.sync.dma_start(
                out=g[:, 4 * lo:4 * (lo + w)].rearrange(
                    "p (r c) -> p r c", r=4),
                in_=cc_out[h].rearrange("r p c -> p r c"))

        # --- main loop: z' = -0.5*|a-b|^2 via K=13 bf16 matmul ------------
        with tc.tile_pool(name="mm", bufs=2, space="PSUM") as mmp, \
             tc.tile_pool(name="ws", bufs=4) as wsp, \
             tc.tile_pool(name="fb", bufs=2) as fbp, \
             tc.tile_pool(name="gb", bufs=2) as gbp:
            for f in range(F):
                for c in range(CH):
                    col = f * CH + c
                    ps = mmp.tile([128, M_PRED], f32, name="mmps", tag="mmps")
                    lhsT = OBSL[:, f * N_OBS + c * 128:
                                f * N_OBS + (c + 1) * 128]
                    for q in range(4):
                        nc.tensor.matmul(
                            ps[:, q * 512:(q + 1) * 512], lhsT=lhsT,
                            rhs=PREDL[:, f * M_PRED + q * 512:
                                      f * M_PRED + (q + 1) * 512],
                            start=True, stop=True)
                    # ACT: stage the whole chunk -> f16 (sole PSUM reader,
                    # so PSUM-free depends on this copy alone)
                    WS = wsp.tile([128, ACT_W], f16, name="WS", tag="WS")
                    nc.scalar.copy(out=WS, in_=ps)
                    # DVE fold tree on the staged f16
                    FB = fbp.tile([128, H1], f16, name="FB", tag="FB")
                    nc.vector.tensor_tensor(
                        out=FB, in0=WS[:, 0:H1], in1=WS[:, H1:ACT_W],
                        op=A.max)
                    GB = gbp.tile([128, 512], f16, name="GB", tag="GB")
                    nc.vector.tensor_tensor(
                        out=GB, in0=FB[:, 0:512], in1=FB[:, 512:1024],
                        op=A.max)
                    # VC slot index: the last frame flushes at cols 123
                    # and 127 (4-wide) so the 3rd gather can launch early.
                    if col < 120:
                        slot = col % TRB
                        flush = TRB if slot == TRB - 1 else 0
                    else:
                        slot = col % 4
                        flush = 4 if slot == 3 else 0
                    nc.vector.tensor_tensor(
                        out=VC[:, slot, :], in0=GB[:, 0:256],
                        in1=GB[:, 256:512], op=A.max)
                    if flush:
                        nc.vector.tensor_reduce(
                            out=zA[:, col - flush + 1:col + 1],
                            in_=VC[:, 0:flush, :], axis=X, op=A.max)
                    if col == 123:
                        gather_part(2)
                if f == 1:
                    gather_part(0)
                elif f == 2:
                    gather_part(1)
            gather_part(3)

        # --- tail: med/mad via value bisection on fp16 residuals ----------
        # sqrt of the first 3/4 of g runs while the last AllGather flies
        # (also triggers the ACT table switch early).
        r16 = pp.tile([128, 512], f16, name="r16", tag="r16")
        nc.scalar.activation(out=r16[:, 0:384], in_=g[:, 0:384], func=AF.Sqrt)
        nc.scalar.activation(out=r16[:, 384:512], in_=g[:, 384:512],
                             func=AF.Sqrt)

        ones16 = pp.tile([128, 128], f16, name="ones16", tag="ones16")
        nc.vector.memset(ones16, 1.0)
        half1 = pp.tile([128, 1], f32, name="half1", tag="half1")
        nc.vector.memset(half1, 0.5)

        acc2 = pp.tile([128, 2], f32, name="acc2", tag="acc2")
        accC = pp.tile([128, 1], f16, name="accC", tag="accC")
        zer256 = pp.tile([128, 256], f16, name="zer256", tag="zer256")
        nc.vector.memset(zer256, 0.0)
        dT = pp.tile([128, 1], f32, name="dT", tag="dT")

        jkA = pp.tile([128, 256], f16, name="jkA", tag="jkA")
        jkD = pp.tile([128, 256], f16, name="jkD", tag="jkD")
        jkD2 = pp.tile([128, 256], f16, name="jkD2", tag="jkD2")

        bp = stack.enter_context(tc.tile_pool(name="bis_ps", bufs=2,
                                              space="PSUM"))

        # Split counting: ACT Sign covers cols [0:256] (sum of sign(T-x)
        # over those 32768 elems = 2*c_act - 32768 + ties), DVE is_lt
        # covers [256:512] (exact count c_dve). Whole-row test
        # count < K=32768 becomes S_act + 2*c_dve < 32768. f16
        # accumulators/ones keep the partition-sum matmul single-pass.
        def bisect(vals, tag, T0):
            Tt = pp.tile([128, 1], f32, name=f"T_{tag}", tag=f"T_{tag}")
            nc.vector.memset(Tt, T0)
            for j in range(N_ITERS):
                step = float(T0 / 2 ** (j + 1))
                tot = bp.tile([128, 1], f32, name=f"tot_{tag}", tag="tot")
                nc.scalar.activation(
                    out=jkA, in_=vals[:, 0:256], func=AF.Sign,
                    bias=Tt[:, 0:1], scale=-1.0, accum_out=acc2[:, 0:1])
                nc.vector.tensor_scalar(
                    out=jkD, in0=vals[:, 256:512], scalar1=Tt[:, 0:1],
                    scalar2=None, op0=A.subtract)
                nc.vector.tensor_scalar(
                    out=jkD2, in0=jkD, scalar1=0.0, scalar2=0.0,
                    op0=A.is_lt, op1=A.add, accum_out=acc2[:, 1:2])
                nc.vector.scalar_tensor_tensor(
                    out=accC, in0=acc2[:, 1:2], scalar=2.0, op0=A.mult,
                    op1=A.add, in1=acc2[:, 0:1])
                nc.tensor.matmul(tot, lhsT=ones16, rhs=accC,
                                 start=True, stop=True)
                nc.vector.tensor_scalar(
                    out=dT, in0=tot, scalar1=32768.0, scalar2=2.0 * step,
                    op0=A.is_lt, op1=A.mult)
                nc.vector.scalar_tensor_tensor(
                    out=Tt, in0=dT, scalar=step, op0=A.subtract, op1=A.add,
                    in1=Tt)
            return Tt

        med = bisect(r16, "med", 2.0)
        negmed = pp.tile([128, 1], f32, name="negmed", tag="negmed")
        nc.vector.tensor_scalar(out=negmed, in0=med, scalar1=-1.0,
                                scalar2=None, op0=A.mult)
        u16 = pp.tile([128, 512], f16, name="u16", tag="u16")
        nc.scalar.activation(out=u16, in_=r16, func=AF.Abs,
                             bias=negmed[:, 0:1], scale=1.0)
        mad = bisect(u16, "mad", 0.5)

        # --- loss = 0.5 * sum(w * d), w = relu(1 - d/(TUNE*std)^2)^2 ------
        c1 = pp.tile([128, 1], f32, name="c1", tag="c1")
        nc.vector.tensor_scalar(out=c1, in0=mad, scalar1=TUNE / MADSTD,
                                scalar2=None, op0=A.mult)
        cs2 = pp.tile([128, 1], f32, name="cs2", tag="cs2")
        nc.vector.tensor_tensor(out=cs2, in0=c1, in1=c1, op=A.mult)
        ncs2 = pp.tile([128, 1], f32, name="ncs2", tag="ncs2")
        nc.vector.tensor_scalar(out=ncs2, in0=cs2, scalar1=-1.0,
                                scalar2=None, op0=A.mult)
        ninv = pp.tile([128, 1], f32, name="ninv", tag="ninv")
        nc.vector.reciprocal(ninv, ncs2)

        # v = relu(1 - d/(TUNE*std)^2) fused: scale is the per-partition
        # -1/(TUNE*std)^2 so the big elementwise multiply never hits DVE
        v = pp.tile([128, 512], f32, name="v", tag="v")
        nc.scalar.activation(out=v, in_=g, func=AF.Relu,
                             bias=1.0, scale=ninv[:, 0:1])
        y = pp.tile([128, 512], f32, name="y", tag="y")
        nc.vector.tensor_tensor(out=y, in0=v, in1=g, op=A.mult)
        S = pp.tile([128, 1], f32, name="S", tag="S")
        jkf = pp.tile([128, 512], f32, name="jkf", tag="jkf")
        nc.vector.scalar_tensor_tensor(
            out=jkf, in0=y, scalar=1.0, op0=A.bypass, op1=A.mult,
            in1=v, accum_out=S)

        ls = bp.tile([1, 1], f32, name="ls")
        nc.tensor.matmul(ls, lhsT=half1, rhs=S, start=True, stop=True)
        ls_sb = pp.tile([1, 1], f32, name="ls_sb", tag="ls_sb")
        nc.scalar.copy(out=ls_sb, in_=ls)
        nc.sync.dma_start(out=out_d, in_=ls_sb)

    from contextlib import ExitStack
    with tile.TileContext(nc) as tc, ExitStack() as stack:
        pp = stack.enter_context(tc.tile_pool(name="persist", bufs=1))
        emit(tc, pp, stack)

    nc.compile()
    return nc


def _split16(x64, dt):
    hi = x64.astype(dt)
    lo = (x64 - hi.astype(np.float64)).astype(dt)
    return hi, lo


def _shard_inputs(points3d_obs, points3d_pred):
    import ml_dtypes
    bf16 = ml_dtypes.bfloat16
    obs = np.asarray(points3d_obs, dtype=np.float32).reshape(BT, N_OBS, 3)
    pred = np.asarray(points3d_pred, dtype=np.float32).reshape(BT, M_PRED, 3)
    in_maps = []
    for core in range(NCORES):
        so = obs[core * F:(core + 1) * F]       # [F, N, 3]
        sp = pred[core * F:(core + 1) * F]      # [F, M, 3]

        ha, la = _split16(so.astype(np.float64), bf16)
        hna, lna = _split16(-0.5 * (so.astype(np.float64) ** 2).sum(-1), bf16)
        hb, lb = _split16(sp.astype(np.float64), bf16)
        hnb, lnb = _split16(-0.5 * (sp.astype(np.float64) ** 2).sum(-1), bf16)

        onesN = np.ones((F, N_OBS), bf16)
        onesM = np.ones((F, M_PRED), bf16)

        # [13, F*N]: hi/lo(-0.5|a|^2), ha, la, ha, 1, 1
        obs_rows = np.stack([
            hna, lna,
            ha[..., 0], ha[..., 1], ha[..., 2],
            la[..., 0], la[..., 1], la[..., 2],
            ha[..., 0], ha[..., 1], ha[..., 2],
            onesN, onesN,
        ], axis=0).reshape(13, F * N_OBS)
        # [13, F*M]: 1, 1, hb, hb, lb, hi/lo(-0.5|b|^2)
        pred_rows = np.stack([
            onesM, onesM,
            hb[..., 0], hb[..., 1], hb[..., 2],
            hb[..., 0], hb[..., 1], hb[..., 2],
            lb[..., 0], lb[..., 1], lb[..., 2],
            hnb, lnb,
        ], axis=0).reshape(13, F * M_PRED)

        in_maps.append({
            "obs_in": np.ascontiguousarray(obs_rows),
            "pred_in": np.ascontiguousarray(pred_rows),
        })
    return in_maps


def _get_nc(stage="D"):
    key = f"nc_{stage}"
    if key not in _CACHE:
        _CACHE[key] = _build_nc(stage)
    return _CACHE[key]


def run(points3d_obs, points3d_pred, stage="D", **kwargs):
    """Run on hardware; kwargs forwarded to run_bass_kernel_spmd."""
    from concourse.bass_utils import run_bass_kernel_spmd
    nc = _get_nc(stage)
    in_maps = _shard_inputs(points3d_obs, points3d_pred)
    res = run_bass_kernel_spmd(nc, in_maps, list(range(NCORES)), **kwargs)
    return res


def kernel(points3d_obs, points3d_pred):
    res = run(points3d_obs, points3d_pred)
    loss = (np.float32(res.results[0]["out"][0, 0])
            + np.float32(res.results[4]["out"][0, 0]))
    return np.asarray(loss, dtype=np.float32).reshape(())
